# revision 42
# baseline (speedup 1.0000x reference)
"""MixedArityTreeLSTM Trainium2 kernel.

Level-synchronous bottom-up Tree-LSTM over B=256 heap-indexed perfect binary
trees (511 nodes, depth 8), E=H=128. Pure data-parallel over 8 NeuronCores
(32 trees per core); all weights replicated.

v13 (~105-108us HW, rel-err ~1.1e-2, gate 2e-2): token/arity-dependent
affine terms for L6..L0 are host-packed via vocab-indexed tables (same
family as the hleaf table):
  wxd[node,g] = (W_g emb[tok] + b_g + d_g*m[node])  -- gathered from a
  [2V, 5, H] table indexed by tok + m*V.  With Ubt' = Ubt - Uun the unary
  term uses UNMASKED child h (exact cancellation for binary nodes):
  psum_g = I@wxd_g + Ubt'_g@(m*h_l) + Ubb_g@(m*h_r) + Uun_g@h_l
so no (1-m) mask stream/op exists anywhere.  A chunk's left-half h is
written by the chain directly into the parent's wx tile block 5, making the
first matmul wave depend only on DMA + child h.  L7 (the DMA-critical
window) instead ships raw x=emb[tok] and computes W@x + b + d*m on device:
an extra DoubleRow pass per gate pairs (W_g|M_g) against (x | (m;1)-rows),
trading ~6us of idle-PE time for 1MB less of first-30us DMA.

Levels 7..4 run in fp8(e4m3) with DoubleRow-paired matmuls (K=256): per gate
one pass (I;Uun_g)@(wxd_g;h_l) + one pass (Ubt'_g;Ubb_g)@(heb;hob), i.e. 2
matmuls/gate instead of 4.  Deep-level fp8 error attenuates through the
forget gates: measured end-to-end rel-err ~8e-3 (vs 4.9e-3 all-bf16, gate
2e-2).  Levels 3..0 stay bf16 single passes, gate-major (PSUM bank sharing).
The leaf level ships host-masked fp8 streams so L7 needs no device mask
work.  L7 ping-pongs two PSUM tag sets; L6..L0 chunks (N=256) alternate two
3-bank sets so matmuls never wait on the previous chunk's activation drain.

The broadcast mask for L6..L0 ships as a [128, 4064] fp8 tensor (DMA'd
after the L7 pieces; gpsimd partition_broadcast was abandoned because its
ucode library load stalled the masks until ~29us).  The fp8 weight pair
packs load flat (a rearranged DMA generated 256B descriptors and starved
the sync queue).  GpSimd only does hob/cob muls at L6/L5 plus the late tail
wxd + weight DMA descgen (emitted after any compute it gates); the serial
chain stays on Vector.  PE warmup feeds off a memset tile so the HAM
clock-gate (1.2->2.4GHz after ~3.5us of activity) opens before work lands.

Every level is stored in BIT-REVERSED node order ("parity layout"), trees
fastest: left children of a level's positions [a, b) sit at the child level's
positions [a, b) and right children at [HALF + a, HALF + b).
"""

import numpy as np
import ml_dtypes

B, D = 256, 8
V, E, H = 32000, 128, 128
NCORES = 8
BL = B // NCORES  # 32 trees per core

LVL_N = {l: BL * (2**l) for l in range(D + 1)}
INT_LEVELS = list(range(D - 1, -1, -1))  # 7..0
FP8_LEVELS = {7, 6, 5, 4}
# mask rows cover levels 6..0 only
MK_OFF = {}
_off = 0
for _l in INT_LEVELS[1:]:
    MK_OFF[_l] = _off
    _off += LVL_N[_l]
MKCOLS = _off  # 4064
MK7_OFF = MKCOLS  # L7 mask appended at the end: [L6..L0 | L7]
MKTOT = MKCOLS + LVL_N[7]  # 8160

# bit-reversal position->node order per level: sig[l][i] = node at position i
SIG = {0: np.array([0])}
for _l in range(1, D + 1):
    SIG[_l] = np.concatenate([2 * SIG[_l - 1], 2 * SIG[_l - 1] + 1])

CPL = {7: 8, 6: 8, 5: 4, 4: 2, 3: 1, 2: 1, 1: 1, 0: 1}
CW = {l: LVL_N[l] // CPL[l] for l in INT_LEVELS}

SEQ = [
    (7, 0), (7, 1), (7, 4), (7, 5), (6, 0), (6, 1), (7, 2), (7, 3),
    (6, 2), (6, 3), (7, 6), (7, 7), (6, 4), (6, 5), (6, 6), (6, 7),
    (5, 0), (5, 1), (5, 2), (5, 3), (4, 0), (4, 1),
    (3, 0), (2, 0), (1, 0), (0, 0),
]
# non-top chunks alternate between the two 3-bank psum tag sets so a chunk's
# matmuls never wait on the previous chunk's psum drain
ODD_TAGS = {c for i, c in enumerate(x for x in SEQ if x[0] != 7) if i % 2 == 1}


def _children(lvl, j):
    """Child chunks (lvl+1, jj) whose h/c this chunk consumes (parity layout)."""
    if lvl == D - 1:
        return []  # children are leaves (host streams)
    N = CW[lvl]
    c0 = j * N
    half = LVL_N[lvl]
    spans = [(c0, c0 + N), (half + c0, half + c0 + N)]
    out = []
    for jj in range(CPL[lvl + 1]):
        a, b = jj * CW[lvl + 1], (jj + 1) * CW[lvl + 1]
        if any(a < hi and b > lo for lo, hi in spans) and (lvl + 1, jj) not in out:
            out.append((lvl + 1, jj))
    return out


BF16 = ml_dtypes.bfloat16

_CACHE = {}

# gate order everywhere: u, i, fl, fr, o  (L7: u, i, o)
# G_POS: slice position in packed ubt/ubb/un8/ub8; G_UUN: position in bf16
# uun pack (i,f,o,u)
G_POS = {"u": 0, "i": 1, "fl": 2, "fr": 3, "o": 4}
G_UUN = {"u": 3, "i": 0, "fl": 1, "fr": None, "o": 2}
GATES_TOP = ["u", "i", "o"]
GATES_INT = ["u", "i", "fl", "fr", "o"]


def _build_nc():
    if "nc" in _CACHE:
        return _CACHE["nc"]

    from contextlib import ExitStack

    import concourse.mybir as mybir
    import concourse.tile as tile
    from concourse import bacc

    dt = mybir.dt
    AF = mybir.ActivationFunctionType
    DR = mybir.MatmulPerfMode.DoubleRow

    nc = bacc.Bacc()

    # bf16 weights pack for tail: ubt(5*128) | ubb(5*128) | uun(4*128) | eye
    wts_d = nc.dram_tensor("wts", [128, 1920], dt.bfloat16, kind="ExternalInput")
    # fp8 pair packs for L7..L4: per gate (I | Uun_g) and (Ubt_g | Ubb_g)
    un8_d = nc.dram_tensor("un8", [128, 5 * 2 * 128], dt.float8e4, kind="ExternalInput")
    ub8_d = nc.dram_tensor("ub8", [128, 5 * 2 * 128], dt.float8e4, kind="ExternalInput")
    # L7 streams: x7 (raw emb) | hleaf_l ; W@x + b + d*m is computed on
    # device (saves 1MB of the critical L7 DMA window vs shipping 3 wxd
    # blocks).  m1 = (m7; ones) rows feed the bias/delta pair slot.
    l7s_d = nc.dram_tensor("l7s", [128, 2 * 4096], dt.float8e4, kind="ExternalInput")
    m1_d = nc.dram_tensor("m1", [2, 4096], dt.float8e4, kind="ExternalInput")
    # L7 pair stationaries: (W_g|M_g) and (0|Uun_g) for gates u,i,o
    wm8_d = nc.dram_tensor("wm8", [128, 3 * 2 * 128], dt.float8e4, kind="ExternalInput")
    uz8_d = nc.dram_tensor("uz8", [128, 3 * 2 * 128], dt.float8e4, kind="ExternalInput")
    # L7 child streams: hlm_l | hlm_r  (2 blocks x 4096, fp8)
    hlm_d = nc.dram_tensor("hlm", [128, 2 * 4096], dt.float8e4, kind="ExternalInput")
    # L6..L4 wxd (fp8): per level, 5 gate blocks x N_l
    WX8_COLS = sum(5 * LVL_N[l] for l in (6, 5, 4))  # 17920
    wx8_d = nc.dram_tensor("wx8", [128, WX8_COLS], dt.float8e4, kind="ExternalInput")
    # L3..L0 wxd (bf16)
    WXI_COLS = sum(5 * LVL_N[l] for l in (3, 2, 1, 0))  # 2400
    wxi_d = nc.dram_tensor("wxi", [128, WXI_COLS], dt.bfloat16, kind="ExternalInput")
    # broadcast mask for L6..L0 (fp8: 0/1 exact, mixes fine with bf16 DVE ins)
    mbc_d = nc.dram_tensor("mbc", [128, MKCOLS], dt.float8e4, kind="ExternalInput")

    h_out_d = nc.dram_tensor("h_out", [H, BL], dt.float32, kind="ExternalOutput")
    c_out_d = nc.dram_tensor("c_out", [H, BL], dt.float32, kind="ExternalOutput")

    WX8_OFF = {}
    _o = 0
    for l in (6, 5, 4):
        WX8_OFF[l] = _o
        _o += 5 * LVL_N[l]
    WXI_OFF = {}
    _o = 0
    for l in (3, 2, 1, 0):
        WXI_OFF[l] = _o
        _o += 5 * LVL_N[l]

    with tile.TileContext(nc) as tc, ExitStack() as ctx:
        consts = ctx.enter_context(tc.tile_pool(name="consts", bufs=1))
        lev = ctx.enter_context(tc.tile_pool(name="lev", bufs=1))

        wts_sb = consts.tile([128, 1920], dt.bfloat16)
        un8_sb = consts.tile([128, 5 * 2 * 128], dt.float8e4)
        ub8_sb = consts.tile([128, 5 * 2 * 128], dt.float8e4)

        def un8g(g):
            return un8_sb[:, G_POS[g] * 256 : (G_POS[g] + 1) * 256].rearrange(
                "p (k h) -> p k h", k=2
            )

        def ub8g(g):
            return ub8_sb[:, G_POS[g] * 256 : (G_POS[g] + 1) * 256].rearrange(
                "p (k h) -> p k h", k=2
            )
        mbc_sb = lev.tile([128, MKCOLS], dt.float8e4, name="mbc", tag="mbc")

        def ubt(g):
            return wts_sb[:, G_POS[g] * 128 : (G_POS[g] + 1) * 128]

        def ubb(g):
            return wts_sb[:, 640 + G_POS[g] * 128 : 640 + (G_POS[g] + 1) * 128]

        def uun(g):
            gi = G_UUN[g]
            return wts_sb[:, 1280 + gi * 128 : 1280 + (gi + 1) * 128]

        eye = wts_sb[:, 1792:1920]

        # SBUF state tiles.  fp8 levels keep wxd + the partner stream in one
        # tile so a strided slice [:, g:T:T-1-g, :] yields the DoubleRow
        # paired moving operand (wxd_g ; partner).
        # L7 stream tile: [x7 | mbx | hleaf_l]; pair1 = [:,0:2,:] (x;mbx),
        # pair3 = [:,1:3,:] (mbx;hl) with stationary (0;Uun)
        l7s_sb = lev.tile([128, 3, 4096], dt.float8e4, name="l7s", tag="l7s")
        wm8_sb = consts.tile([128, 3 * 2 * 128], dt.float8e4)
        uz8_sb = consts.tile([128, 3 * 2 * 128], dt.float8e4)

        def wm8g(gp):
            return wm8_sb[:, gp * 256 : (gp + 1) * 256].rearrange(
                "p (k h) -> p k h", k=2
            )

        def uz8g(gp):
            return uz8_sb[:, gp * 256 : (gp + 1) * 256].rearrange(
                "p (k h) -> p k h", k=2
            )
        hlm_sb = lev.tile([128, 2, 4096], dt.float8e4, name="hlm", tag="hlm")
        wx_t = {}
        for l in (6, 5, 4):
            wx_t[l] = lev.tile(
                [128, 6, LVL_N[l]], dt.float8e4, name=f"wx{l}", tag=f"wx{l}"
            )
        for l in (3, 2, 1, 0):
            wx_t[l] = lev.tile(
                [128, 6, LVL_N[l]], dt.bfloat16, name=f"wx{l}", tag=f"wx{l}"
            )
        # h storage: a chunk's left-half h is written straight into the
        # parent level's wx tile block 5 (the pair-1 partner stream);
        # right-half h goes to hr_t[lvl].  c keeps per-level tiles.
        c_t = {}
        hr_t = {}
        for lvl in INT_LEVELS:
            n = LVL_N[lvl]
            hdt = dt.float32 if lvl == 0 else dt.bfloat16
            c_t[lvl] = lev.tile([H, n], hdt, name=f"c_l{lvl}", tag=f"c_l{lvl}")
            if lvl >= 1:
                hr_t[lvl] = lev.tile(
                    [H, n // 2], dt.bfloat16, name=f"hr{lvl}", tag=f"hr{lvl}"
                )
        h0_t = lev.tile([H, BL], dt.float32, name="h_l0", tag="h_l0")


        # ---------------- DMA schedule ----------------
        # sync HWDGE: mkb, weights, then hlm + wx7(u,i) in chunk-need order.
        # scalar HWDGE: wx7(o,hlu) pieces, then wx8 L6/L5/L4.
        # gpsimd SWDGE: only the small late tail wxd (descgen after the
        # broadcasts so the gpsimd engine is free when compute needs it).
        nc.sync.dma_start(out=ub8_sb, in_=ub8_d[:, :])
        nc.sync.dma_start(out=wm8_sb, in_=wm8_d[:, :])
        nc.scalar.dma_start(out=un8_sb, in_=un8_d[:, :])
        nc.scalar.dma_start(out=uz8_sb, in_=uz8_d[:, :])
        # mbx block: memset (kill NaN garbage: only rows 0-1 are real; the
        # paired stationary rows 2-127 are zero) then the (m7; ones) rows.
        # On GpSimd so the Vector warmup-feed memset is not delayed.
        nc.gpsimd.memset(l7s_sb[:, 1, :], 0.0)
        nc.sync.dma_start(out=l7s_sb[0:2, 1, :], in_=m1_d[:, :])

        # 1024-col pieces; group order matches SEQ: (7,0),(7,1) then (7,4),(7,5)
        # ... ; the 6 L7 streams are spread 2-per-queue across all 3 queues.
        for a in (0, 2048, 1024, 3072):
            b = a + 1024
            nc.sync.dma_start(out=hlm_sb[:, 0, a:b], in_=hlm_d[:, a:b])
            nc.sync.dma_start(out=hlm_sb[:, 1, a:b], in_=hlm_d[:, 4096 + a : 4096 + b])
            nc.scalar.dma_start(out=l7s_sb[:, 0, a:b], in_=l7s_d[:, a:b])
            nc.scalar.dma_start(
                out=l7s_sb[:, 2, a:b], in_=l7s_d[:, 4096 + a : 4096 + b]
            )
        # broadcast mask: not needed until L6 masks (~30us), after L7 pieces
        nc.sync.dma_start(out=mbc_sb[:, 0:2048], in_=mbc_d[:, 0:2048])
        nc.sync.dma_start(out=mbc_sb[:, 2048:MKCOLS], in_=mbc_d[:, 2048:MKCOLS])
        WX8_Q = {6: (0, 2), 5: (0, 1), 4: (0, 1, 2, 3, 4)}  # gates on sync
        for l in (6, 5, 4):
            n = LVL_N[l]
            base = WX8_OFF[l]
            for g in range(5):
                eng = nc.sync if g in WX8_Q[l] else nc.scalar
                eng.dma_start(
                    out=wx_t[l][:, g, :],
                    in_=wx8_d[:, base + g * n : base + (g + 1) * n],
                )

        # ---------------- PSUM + pools ----------------
        psum = ctx.enter_context(tc.tile_pool(name="psum", bufs=1, space="PSUM"))
        work = ctx.enter_context(tc.tile_pool(name="work", bufs=4))

        # PE warmup: fed by memset tile (no DMA dependency), ~8 cold matmuls
        # ~= 5us of activity so the HAM clock gate is open when work lands.
        warm_in = consts.tile([128, 512], dt.bfloat16)
        nc.vector.memset(warm_in, 0.0)
        warm = psum.tile([H, 512], dt.float32, tag="tU", name="warm")
        for _ in range(10):
            nc.tensor.matmul(warm, warm_in[:, 0:128], warm_in, start=True, stop=True)

        # tail wxd + bf16 weight pack DMAs (needed only from ~L3): their
        # SWDGE descgen runs on the gpsimd engine AFTER the broadcasts so
        # the masks are never blocked behind descriptor generation
        for l in (3, 2, 1, 0):
            n = LVL_N[l]
            base = WXI_OFF[l]
            for g in range(5):
                nc.gpsimd.dma_start(
                    out=wx_t[l][:, g, :],
                    in_=wxi_d[:, base + g * n : base + (g + 1) * n],
                )
        nc.gpsimd.dma_start(out=wts_sb, in_=wts_d[:, :])

        state = {}

        def phase_masks(lvl, j):
            if lvl == D - 1:
                return
            N = CW[lvl]
            c0 = j * N
            cch = c_t[lvl + 1]
            moff = MK_OFF[lvl] + c0
            mb = mbc_sb[:, moff : moff + N]
            hdt = dt.float8e4 if lvl in FP8_LEVELS else dt.bfloat16
            # gpsimd muls are slow (~1.1us); keep them only where the psum
            # runway covers them (big interleaved levels)
            eng2 = nc.gpsimd if lvl >= 5 else nc.vector

            h_e = wx_t[lvl][:, 5, c0 : c0 + N]       # child left-half h
            h_o = hr_t[lvl + 1][:, c0 : c0 + N]      # child right-half h
            hm = work.tile([128, 2, N], hdt, tag="hm", name="hm")
            nc.vector.tensor_mul(hm[:, 0, :], h_e, mb)
            eng2.tensor_mul(hm[:, 1, :], h_o, mb)
            cob = work.tile([128, N], dt.bfloat16, tag="cob", name="cob")
            eng2.tensor_mul(cob, cch[:, LVL_N[lvl] + c0 : LVL_N[lvl] + c0 + N], mb)
            state[(lvl, j)] = {
                "hm": hm,
                "cob": cob,
                "c_e": cch[:, c0 : c0 + N],
            }

        def phase_body(lvl, j):
            N = CW[lvl]
            c0 = j * N
            top = lvl == D - 1
            fp8 = lvl in FP8_LEVELS
            st = state.setdefault((lvl, j), {})
            odd = (lvl, j) in ODD_TAGS
            tagA = "tA" if odd else "bgA"
            tagB = "tB" if odd else "bgB"
            tagU = "tU" if odd else "bgu"
            if top and j % 2 == 1:
                # L7 uses only pA+pU; odd chunks ping-pong onto the banks
                # that pB/the tail set would otherwise leave idle here
                tagA, tagU = "bgB", "tU"

            gates = GATES_TOP if top else GATES_INT
            # pad psum tiles to full banks so odd/big sets never share a bank
            pA = psum.tile(
                [H, max(2 * N, 512)], dt.float32, tag=tagA, name=f"pA{lvl}_{j}"
            )[:, 0 : 2 * N]
            pB = (
                None
                if top
                else psum.tile(
                    [H, max(2 * N, 512)], dt.float32, tag=tagB, name=f"pB{lvl}_{j}"
                )[:, 0 : 2 * N]
            )
            pU = psum.tile([H, max(N, 512)], dt.float32, tag=tagU, name=f"pU{lvl}_{j}")[
                :, 0:N
            ]
            sl = {"u": pU}
            if top:
                sl["i"], sl["o"] = pA[:, 0:N], pA[:, N : 2 * N]
            else:
                sl["i"], sl["fl"] = pA[:, 0:N], pA[:, N : 2 * N]
                sl["fr"], sl["o"] = pB[:, 0:N], pB[:, N : 2 * N]

            def act(g):
                if g == "u":
                    gu = work.tile([128, N], dt.bfloat16, tag="gu", name="gu")
                    nc.scalar.activation(gu, pU, AF.Tanh)
                    st["gu"] = gu
                elif (top and g == "o") or (not top and g == "fl"):
                    gAB = work.tile([128, 2 * N], dt.bfloat16, tag="gAB", name="gAB")
                    nc.scalar.activation(gAB, pA, AF.Sigmoid)
                    if top:
                        st["gi"], st["go"] = gAB[:, 0:N], gAB[:, N : 2 * N]
                    else:
                        st["gi"], st["gfl"] = gAB[:, 0:N], gAB[:, N : 2 * N]
                elif not top and g == "o":
                    gFO = work.tile([128, 2 * N], dt.bfloat16, tag="gFO", name="gFO")
                    nc.scalar.activation(gFO, pB, AF.Sigmoid)
                    st["gfr"], st["go"] = gFO[:, 0:N], gFO[:, N : 2 * N]

            if fp8:
                # paired DoubleRow passes: (I;Uun_g)@(wxd_g;partner) then
                # (Ubt_g;Ubb_g)@(heb;hob)
                if top:
                    hm = hlm_sb[:, :, c0 : c0 + N]
                    gpos = {"u": 0, "i": 1, "o": 2}
                else:
                    T = 6
                    wxt = wx_t[lvl]
                    hm = st["hm"]
                    gpos = G_POS
                def pair1(g):
                    if top:
                        nc.tensor.matmul(
                            sl[g], wm8g(gpos[g]), l7s_sb[:, 0:2, c0 : c0 + N],
                            start=True, stop=False, perf_mode=DR,
                        )
                        nc.tensor.matmul(
                            sl[g], uz8g(gpos[g]), l7s_sb[:, 1:3, c0 : c0 + N],
                            start=False, stop=False, perf_mode=DR,
                        )
                        return
                    gp = gpos[g]
                    pair = wxt[:, gp : T : T - 1 - gp, c0 : c0 + N]
                    nc.tensor.matmul(
                        sl[g], un8g(g), pair, start=True, stop=False,
                        perf_mode=DR,
                    )

                def pair2(g):
                    nc.tensor.matmul(
                        sl[g], ub8g(g), hm, start=False, stop=True,
                        perf_mode=DR,
                    )
                    act(g)

                if N == 512:
                    # gate slices are bank-aligned: full runway legal
                    for g in gates:
                        pair1(g)
                    for g in gates:
                        pair2(g)
                else:
                    # pA/pB slices share a bank: emit bank-disjoint waves
                    for grp in (("u", "i"), ("fl", "fr"), ("o",)):
                        for g in grp:
                            pair1(g)
                        for g in grp:
                            pair2(g)
            else:
                hm = st["hm"]
                heb, hob = hm[:, 0, :], hm[:, 1, :]
                h_e = wx_t[lvl][:, 5, c0 : c0 + N]
                wxs = {g: wx_t[lvl][:, G_POS[g], c0 : c0 + N] for g in gates}
                # tail chunks share PSUM banks between gate slices -> strictly
                # gate-major (one open accumulation group per bank at a time)
                for g in gates:
                    ps = sl[g]
                    nc.tensor.matmul(ps, eye, wxs[g], start=True, stop=False)
                    nc.tensor.matmul(ps, ubt(g), heb, start=False, stop=False)
                    last = G_UUN[g] is None
                    nc.tensor.matmul(ps, ubb(g), hob, start=False, stop=last)
                    if not last:
                        nc.tensor.matmul(ps, uun(g), h_e, start=False, stop=True)
                    act(g)

        def phase_chain(lvl, j):
            N = CW[lvl]
            c0 = j * N
            top = lvl == D - 1
            st = state.pop((lvl, j))
            cs = c_t[lvl][:, c0 : c0 + N]
            wdt = dt.float32 if lvl == 0 else dt.bfloat16
            if top:
                nc.vector.tensor_mul(cs, st["gi"], st["gu"])
            else:
                t1 = work.tile([128, N], wdt, tag="t1", name="t1")
                nc.vector.tensor_mul(t1, st["gi"], st["gu"])
                t2 = work.tile([128, N], wdt, tag="t2", name="t2")
                nc.vector.tensor_mul(t2, st["gfl"], st["c_e"])
                t3 = work.tile([128, N], wdt, tag="t3", name="t3")
                nc.vector.tensor_mul(t3, st["gfr"], st["cob"])
                nc.vector.tensor_add(cs, t1, t2)
                nc.vector.tensor_add(cs, cs, t3)
            tch = work.tile([128, N], wdt, tag="tch", name="tch")
            nc.scalar.activation(tch, cs, AF.Tanh)
            # h destinations: left-half columns -> parent wx block 5
            # (pair-1 partner); right-half -> hr_t[lvl]; root -> h0
            if lvl == 0:
                nc.vector.tensor_mul(h0_t[:, c0 : c0 + N], st["go"], tch)
            else:
                half = LVL_N[lvl] // 2
                if c0 + N <= half:
                    nc.vector.tensor_mul(
                        wx_t[lvl - 1][:, 5, c0 : c0 + N], st["go"], tch
                    )
                elif c0 >= half:
                    nc.vector.tensor_mul(
                        hr_t[lvl][:, c0 - half : c0 - half + N], st["go"], tch
                    )
                else:
                    nl = half - c0
                    nc.vector.tensor_mul(
                        wx_t[lvl - 1][:, 5, c0 : c0 + nl],
                        st["go"][:, 0:nl], tch[:, 0:nl],
                    )
                    nc.vector.tensor_mul(
                        hr_t[lvl][:, 0 : N - nl],
                        st["go"][:, nl:N], tch[:, nl:N],
                    )

        pending = []
        for seq_i, (lvl, j) in enumerate(SEQ):
            if seq_i == 2:
                # keep-warm burst: the first L7 chunks finish ~3-7us before
                # the next pieces land; without activity the HAM clock gate
                # drops the PE back to 1.2GHz mid-level
                warm2 = psum.tile([H, 512], dt.float32, tag="tU", name="warm2")
                for _ in range(6):
                    nc.tensor.matmul(
                        warm2, warm_in[:, 0:128], warm_in, start=True, stop=True
                    )
            for ch in _children(lvl, j):
                if ch in pending:
                    phase_chain(*ch)
                    pending.remove(ch)
            phase_masks(lvl, j)
            phase_body(lvl, j)
            pending.append((lvl, j))
            while len(pending) > 1:
                phase_chain(*pending.pop(0))
        for ch in pending:
            phase_chain(*ch)

        # output: 256 partition-descriptors split across all three queues
        nc.sync.dma_start(out=h_out_d[0:86, :], in_=h0_t[0:86, :BL])
        nc.scalar.dma_start(out=c_out_d[0:86, :], in_=c_t[0][0:86, :BL])
        nc.gpsimd.dma_start(out=h_out_d[86:128, :], in_=h0_t[86:128, :BL])
        nc.gpsimd.dma_start(out=c_out_d[86:128, :], in_=c_t[0][86:128, :BL])

    nc.finalize()
    _CACHE["nc"] = nc
    return nc


def prep_shared_inputs(emb, W, bW, Ubin, bUbin, Uun, bUun):
    import concourse.mybir as mybir

    F8 = np.dtype(mybir.dt.np(mybir.dt.float8e4))

    emb = np.asarray(emb, np.float32)
    W = np.asarray(W, np.float32)
    bW = np.asarray(bW, np.float32)
    Ubin = np.asarray(Ubin, np.float32)
    bUbin = np.asarray(bUbin, np.float32)
    Uun = np.asarray(Uun, np.float32)
    bUun = np.asarray(bUun, np.float32)

    # gate order u, i, fl, fr, o ; bias b (unary) and b+d (binary)
    b_rows = np.stack(
        [
            bW[3] + bUun[3],      # u
            bW[0] + bUun[0],      # i
            bW[1] + bUun[1],      # fl
            bW[1] + bUbin[2],     # fr (same either arity; unary killed via cob)
            bW[2] + bUun[2],      # o
        ]
    )
    bd_rows = np.stack(
        [
            bW[3] + bUbin[4],
            bW[0] + bUbin[0],
            bW[1] + bUbin[1],
            bW[1] + bUbin[2],
            bW[2] + bUbin[3],
        ]
    )
    Wg = np.stack([W[3], W[0], W[1], W[1], W[2]])  # u,i,fl,fr,o

    # [2V, 5, H] combined tables indexed by tok + m*V (m=1 -> binary biases)
    wx = np.einsum("ve,geh->vgh", emb, Wg, optimize=True)
    tab_bf = np.empty((2 * V, 5, H), dtype=BF16)
    tab_bf[:V] = (wx + b_rows[None, :, :]).astype(BF16)
    tab_bf[V:] = (wx + bd_rows[None, :, :]).astype(BF16)
    tab_f8 = tab_bf.astype(np.float32).astype(F8)

    hleaf_tab = np.tanh(emb @ W[3] + bW[3]).astype(F8)
    emb_f8 = emb.astype(F8)

    # Ubt' = Ubt - Uun so the unary term can use UNMASKED child h:
    # binary cols:  Ubt'@h + Uun@h = Ubt@h ;  unary cols: Ubt'@0 + Uun@h
    ub_order = [4, 0, 1, 2, 3]  # Ubin gate index for u,i,fl,fr,o
    uun_of = [Uun[3], Uun[0], Uun[1], None, Uun[2]]
    ubt_p = np.concatenate(
        [
            Ubin[g][:128] - (u if u is not None else 0)
            for g, u in zip(ub_order, uun_of)
        ],
        axis=1,
    )
    ubb_p = np.concatenate([Ubin[g][128:] for g in ub_order], axis=1)
    uun_p = np.concatenate([Uun[g] for g in range(4)], axis=1)
    eye = np.eye(128, dtype=np.float32)
    wts = np.concatenate([ubt_p, ubb_p, uun_p, eye], axis=1).astype(BF16)

    # fp8 pair packs: per gate (I | Uun_g) and (Ubt_g | Ubb_g), [128, 5*2*128]
    uun_g = {"u": Uun[3], "i": Uun[0], "fl": Uun[1], "fr": np.zeros_like(eye),
             "o": Uun[2]}
    un8 = np.concatenate(
        [np.concatenate([eye, uun_g[g]], axis=1) for g in GATES_INT], axis=1
    ).astype(F8)
    ub8 = np.concatenate(
        [
            np.concatenate(
                [Ubin[gi][:128] - (u if u is not None else 0), Ubin[gi][128:]],
                axis=1,
            )
            for gi, u in zip(ub_order, uun_of)
        ],
        axis=1,
    ).astype(F8)

    # L7 pair stationaries for gates u,i,o: (W_g | M_g) and (0 | Uun_g),
    # where M_g rows 0/1 are the arity delta d_g and unary bias b_g
    wm_parts = []
    uz_parts = []
    for gi, wg, ug in ((0, W[3], Uun[3]), (1, W[0], Uun[0]), (4, W[2], Uun[2])):
        M = np.zeros((128, 128), np.float32)
        M[0] = bd_rows[gi] - b_rows[gi]
        M[1] = b_rows[gi]
        wm_parts.append(np.concatenate([Wg[gi], M], axis=1))
        uz_parts.append(
            np.concatenate([np.zeros((128, 128), np.float32), ug], axis=1)
        )
    wm8 = np.concatenate(wm_parts, axis=1).astype(F8)
    uz8 = np.concatenate(uz_parts, axis=1).astype(F8)

    return dict(
        _tab_bf=tab_bf, _tab_f8=tab_f8, _hleaf=hleaf_tab, _f8=F8, _emb8=emb_f8,
        wts=np.ascontiguousarray(wts), un8=np.ascontiguousarray(un8),
        ub8=np.ascontiguousarray(ub8), wm8=np.ascontiguousarray(wm8),
        uz8=np.ascontiguousarray(uz8),
    )


def prep_core_inputs(tokens_c, arity_c, shared):
    """Per-core inputs: gather vocab tables into feature-major streams.

    Each level is packed in bit-reversed node order, trees fastest
    (col = position * BL + tree).
    """
    tokens_c = np.asarray(tokens_c)
    arity_c = np.asarray(arity_c, np.int64)
    tab_bf = shared["_tab_bf"]
    tab_f8 = shared["_tab_f8"]
    hleaf_tab = shared["_hleaf"]
    F8 = shared["_f8"]

    def lvl_toks(l):
        off = 2**l - 1
        return tokens_c[:, off + SIG[l]].T.reshape(-1)

    def lvl_mask(l):
        off = 2**l - 1
        return (arity_c[:, off + SIG[l]].T.reshape(-1) == 1)

    # L7 streams: raw emb x7 | raw hleaf_l ; masked hlm_l/hlm_r; m1 rows
    t7 = lvl_toks(7)
    m7 = lvl_mask(7)
    leaf_toks = lvl_toks(8)
    hl = hleaf_tab[leaf_toks]  # [8192, H] fp8
    hl_l, hl_r = hl[:4096], hl[4096:]
    m7f = m7[:, None]
    z8 = np.zeros((1, 1), dtype=F8)
    l7s = np.concatenate([shared["_emb8"][t7].T, hl_l.T], axis=1)
    m1 = np.stack([m7.astype(np.float32), np.ones(4096, np.float32)]).astype(F8)
    hlm = np.concatenate(
        [np.where(m7f, hl_l, z8).T, np.where(m7f, hl_r, z8).T], axis=1
    )

    # internal levels: fp8 for 6..4, bf16 for 3..0
    wx8_cols = []
    wxi_cols = []
    mrows = []
    for l in range(6, -1, -1):
        tl = lvl_toks(l)
        ml = lvl_mask(l)
        if l >= 4:
            blk = tab_f8[tl + ml * V].transpose(1, 2, 0)  # [5, H, N_l]
            wx8_cols.append(np.concatenate(list(blk), axis=1))
        else:
            blk = tab_bf[tl + ml * V].transpose(1, 2, 0)
            wxi_cols.append(np.concatenate(list(blk), axis=1))
        mrows.append(ml.astype(BF16))
    wx8 = np.ascontiguousarray(np.concatenate(wx8_cols, axis=1))
    wxi = np.ascontiguousarray(np.concatenate(wxi_cols, axis=1))
    mrow = np.concatenate(mrows).astype(np.float32)

    out = {k: v for k, v in shared.items() if not k.startswith("_")}
    out.update(
        l7s=np.ascontiguousarray(l7s),
        m1=np.ascontiguousarray(m1),
        hlm=np.ascontiguousarray(hlm),
        wx8=wx8,
        wxi=wxi,
        mbc=np.ascontiguousarray(np.broadcast_to(mrow, (128, MKCOLS)).astype(F8)),
    )
    return out


def kernel(tokens, arity, emb, W, bW, Ubin, bUbin, Uun, bUun):
    from concourse.bass_utils import run_bass_kernel_spmd

    tokens = np.asarray(tokens)
    arity = np.asarray(arity)

    shared = prep_shared_inputs(emb, W, bW, Ubin, bUbin, Uun, bUun)
    in_maps = [
        prep_core_inputs(
            tokens[k * BL : (k + 1) * BL], arity[k * BL : (k + 1) * BL], shared
        )
        for k in range(NCORES)
    ]

    nc = _build_nc()
    res = run_bass_kernel_spmd(nc, in_maps, core_ids=list(range(NCORES)))
    results = res.results

    h = np.concatenate([r["h_out"].T for r in results], axis=0)
    c = np.concatenate([r["c_out"].T for r in results], axis=0)
    return h.astype(np.float32), c.astype(np.float32)


# revision 43
# speedup vs baseline: 1.0257x; 1.0257x over previous
"""MixedArityTreeLSTM Trainium2 kernel.

Level-synchronous bottom-up Tree-LSTM over B=256 heap-indexed perfect binary
trees (511 nodes, depth 8), E=H=128. Pure data-parallel over 8 NeuronCores
(32 trees per core); all weights replicated.

v13 (~105-108us HW, rel-err ~1.1e-2, gate 2e-2): token/arity-dependent
affine terms for L6..L0 are host-packed via vocab-indexed tables (same
family as the hleaf table):
  wxd[node,g] = (W_g emb[tok] + b_g + d_g*m[node])  -- gathered from a
  [2V, 5, H] table indexed by tok + m*V.  With Ubt' = Ubt - Uun the unary
  term uses UNMASKED child h (exact cancellation for binary nodes):
  psum_g = I@wxd_g + Ubt'_g@(m*h_l) + Ubb_g@(m*h_r) + Uun_g@h_l
so no (1-m) mask stream/op exists anywhere.  A chunk's left-half h is
written by the chain directly into the parent's wx tile block 5, making the
first matmul wave depend only on DMA + child h.  L7 (the DMA-critical
window) instead ships raw x=emb[tok] and computes W@x + b + d*m on device:
an extra DoubleRow pass per gate pairs (W_g|M_g) against (x | (m;1)-rows),
trading ~6us of idle-PE time for 1MB less of first-30us DMA.

Levels 7..4 run in fp8(e4m3) with DoubleRow-paired matmuls (K=256): per gate
one pass (I;Uun_g)@(wxd_g;h_l) + one pass (Ubt'_g;Ubb_g)@(heb;hob), i.e. 2
matmuls/gate instead of 4.  Deep-level fp8 error attenuates through the
forget gates: measured end-to-end rel-err ~8e-3 (vs 4.9e-3 all-bf16, gate
2e-2).  Levels 3..0 stay bf16 single passes, gate-major (PSUM bank sharing).
The leaf level ships host-masked fp8 streams so L7 needs no device mask
work.  L7 ping-pongs two PSUM tag sets; L6..L0 chunks (N=256) alternate two
3-bank sets so matmuls never wait on the previous chunk's activation drain.

The broadcast mask for L6..L0 ships as a [128, 4064] fp8 tensor (DMA'd
after the L7 pieces; gpsimd partition_broadcast was abandoned because its
ucode library load stalled the masks until ~29us).  The fp8 weight pair
packs load flat (a rearranged DMA generated 256B descriptors and starved
the sync queue).  GpSimd only does hob/cob muls at L6/L5 plus the late tail
wxd + weight DMA descgen (emitted after any compute it gates); the serial
chain stays on Vector.  PE warmup feeds off a memset tile so the HAM
clock-gate (1.2->2.4GHz after ~3.5us of activity) opens before work lands.

Every level is stored in BIT-REVERSED node order ("parity layout"), trees
fastest: left children of a level's positions [a, b) sit at the child level's
positions [a, b) and right children at [HALF + a, HALF + b).
"""

import numpy as np
import ml_dtypes

B, D = 256, 8
V, E, H = 32000, 128, 128
NCORES = 8
BL = B // NCORES  # 32 trees per core

LVL_N = {l: BL * (2**l) for l in range(D + 1)}
INT_LEVELS = list(range(D - 1, -1, -1))  # 7..0
FP8_LEVELS = {7, 6, 5, 4}
# mask rows cover levels 6..0 only
MK_OFF = {}
_off = 0
for _l in INT_LEVELS[1:]:
    MK_OFF[_l] = _off
    _off += LVL_N[_l]
MKCOLS = _off  # 4064
MK7_OFF = MKCOLS  # L7 mask appended at the end: [L6..L0 | L7]
MKTOT = MKCOLS + LVL_N[7]  # 8160

# bit-reversal position->node order per level: sig[l][i] = node at position i
SIG = {0: np.array([0])}
for _l in range(1, D + 1):
    SIG[_l] = np.concatenate([2 * SIG[_l - 1], 2 * SIG[_l - 1] + 1])

CPL = {7: 8, 6: 8, 5: 4, 4: 2, 3: 1, 2: 1, 1: 1, 0: 1}
CW = {l: LVL_N[l] // CPL[l] for l in INT_LEVELS}

SEQ = [
    (7, 0), (7, 1), (7, 4), (7, 5), (6, 0), (6, 1), (7, 2), (7, 3),
    (6, 2), (6, 3), (7, 6), (7, 7), (6, 4), (6, 5), (6, 6), (6, 7),
    (5, 0), (5, 1), (5, 2), (5, 3), (4, 0), (4, 1),
    (3, 0), (2, 0), (1, 0), (0, 0),
]
# non-top chunks alternate between the two 3-bank psum tag sets so a chunk's
# matmuls never wait on the previous chunk's psum drain
ODD_TAGS = {c for i, c in enumerate(x for x in SEQ if x[0] != 7) if i % 2 == 1}


def _children(lvl, j):
    """Child chunks (lvl+1, jj) whose h/c this chunk consumes (parity layout)."""
    if lvl == D - 1:
        return []  # children are leaves (host streams)
    N = CW[lvl]
    c0 = j * N
    half = LVL_N[lvl]
    spans = [(c0, c0 + N), (half + c0, half + c0 + N)]
    out = []
    for jj in range(CPL[lvl + 1]):
        a, b = jj * CW[lvl + 1], (jj + 1) * CW[lvl + 1]
        if any(a < hi and b > lo for lo, hi in spans) and (lvl + 1, jj) not in out:
            out.append((lvl + 1, jj))
    return out


BF16 = ml_dtypes.bfloat16

_CACHE = {}

# gate order everywhere: u, i, fl, fr, o  (L7: u, i, o)
# G_POS: slice position in packed ubt/ubb/un8/ub8; G_UUN: position in bf16
# uun pack (i,f,o,u)
G_POS = {"u": 0, "i": 1, "fl": 2, "fr": 3, "o": 4}
G_UUN = {"u": 3, "i": 0, "fl": 1, "fr": None, "o": 2}
GATES_TOP = ["u", "i", "o"]
GATES_INT = ["u", "i", "fl", "fr", "o"]


def _build_nc():
    if "nc" in _CACHE:
        return _CACHE["nc"]

    from contextlib import ExitStack

    import concourse.mybir as mybir
    import concourse.tile as tile
    from concourse import bacc

    dt = mybir.dt
    AF = mybir.ActivationFunctionType
    DR = mybir.MatmulPerfMode.DoubleRow

    nc = bacc.Bacc()

    # bf16 weights pack for tail: ubt(5*128) | ubb(5*128) | uun(4*128) | eye
    wts_d = nc.dram_tensor("wts", [128, 1920], dt.bfloat16, kind="ExternalInput")
    # fp8 pair packs for L7..L4: per gate (I | Uun_g) and (Ubt_g | Ubb_g)
    un8_d = nc.dram_tensor("un8", [128, 5 * 2 * 128], dt.float8e4, kind="ExternalInput")
    ub8_d = nc.dram_tensor("ub8", [128, 5 * 2 * 128], dt.float8e4, kind="ExternalInput")
    # L7 streams: x7 (raw emb) | hleaf_l ; W@x + b + d*m is computed on
    # device (saves 1MB of the critical L7 DMA window vs shipping 3 wxd
    # blocks).  m1 = (m7; ones) rows feed the bias/delta pair slot.
    l7s_d = nc.dram_tensor("l7s", [128, 2 * 4096], dt.float8e4, kind="ExternalInput")
    m1_d = nc.dram_tensor("m1", [2, 4096], dt.float8e4, kind="ExternalInput")
    # L7 pair stationaries: (W_g|M_g) and (0|Uun_g) for gates u,i,o
    wm8_d = nc.dram_tensor("wm8", [128, 3 * 2 * 128], dt.float8e4, kind="ExternalInput")
    uz8_d = nc.dram_tensor("uz8", [128, 3 * 2 * 128], dt.float8e4, kind="ExternalInput")
    # L7 child streams: hlm_l | hlm_r  (2 blocks x 4096, fp8)
    hlm_d = nc.dram_tensor("hlm", [128, 2 * 4096], dt.float8e4, kind="ExternalInput")
    # L6..L4 wxd (fp8): per level, 5 gate blocks x N_l
    WX8_COLS = sum(5 * LVL_N[l] for l in (6, 5, 4))  # 17920
    wx8_d = nc.dram_tensor("wx8", [128, WX8_COLS], dt.float8e4, kind="ExternalInput")
    # L3..L0 wxd (bf16)
    WXI_COLS = sum(5 * LVL_N[l] for l in (3, 2, 1, 0))  # 2400
    wxi_d = nc.dram_tensor("wxi", [128, WXI_COLS], dt.bfloat16, kind="ExternalInput")
    # broadcast mask for L6..L0 (fp8: 0/1 exact, mixes fine with bf16 DVE ins)
    mbc_d = nc.dram_tensor("mbc", [128, MKCOLS], dt.float8e4, kind="ExternalInput")

    h_out_d = nc.dram_tensor("h_out", [H, BL], dt.float32, kind="ExternalOutput")
    c_out_d = nc.dram_tensor("c_out", [H, BL], dt.float32, kind="ExternalOutput")

    WX8_OFF = {}
    _o = 0
    for l in (6, 5, 4):
        WX8_OFF[l] = _o
        _o += 5 * LVL_N[l]
    WXI_OFF = {}
    _o = 0
    for l in (3, 2, 1, 0):
        WXI_OFF[l] = _o
        _o += 5 * LVL_N[l]

    with tile.TileContext(nc) as tc, ExitStack() as ctx:
        consts = ctx.enter_context(tc.tile_pool(name="consts", bufs=1))
        lev = ctx.enter_context(tc.tile_pool(name="lev", bufs=1))

        wts_sb = consts.tile([128, 1920], dt.bfloat16)
        un8_sb = consts.tile([128, 5 * 2 * 128], dt.float8e4)
        ub8_sb = consts.tile([128, 5 * 2 * 128], dt.float8e4)

        def un8g(g):
            return un8_sb[:, G_POS[g] * 256 : (G_POS[g] + 1) * 256].rearrange(
                "p (k h) -> p k h", k=2
            )

        def ub8g(g):
            return ub8_sb[:, G_POS[g] * 256 : (G_POS[g] + 1) * 256].rearrange(
                "p (k h) -> p k h", k=2
            )
        mbc_sb = lev.tile([128, MKCOLS], dt.float8e4, name="mbc", tag="mbc")

        def ubt(g):
            return wts_sb[:, G_POS[g] * 128 : (G_POS[g] + 1) * 128]

        def ubb(g):
            return wts_sb[:, 640 + G_POS[g] * 128 : 640 + (G_POS[g] + 1) * 128]

        def uun(g):
            gi = G_UUN[g]
            return wts_sb[:, 1280 + gi * 128 : 1280 + (gi + 1) * 128]

        eye = wts_sb[:, 1792:1920]

        # SBUF state tiles.  fp8 levels keep wxd + the partner stream in one
        # tile so a strided slice [:, g:T:T-1-g, :] yields the DoubleRow
        # paired moving operand (wxd_g ; partner).
        # L7 stream tile: [x7 | mbx | hleaf_l]; pair1 = [:,0:2,:] (x;mbx),
        # pair3 = [:,1:3,:] (mbx;hl) with stationary (0;Uun)
        l7s_sb = lev.tile([128, 3, 4096], dt.float8e4, name="l7s", tag="l7s")
        wm8_sb = consts.tile([128, 3 * 2 * 128], dt.float8e4)
        uz8_sb = consts.tile([128, 3 * 2 * 128], dt.float8e4)

        def wm8g(gp):
            return wm8_sb[:, gp * 256 : (gp + 1) * 256].rearrange(
                "p (k h) -> p k h", k=2
            )

        def uz8g(gp):
            return uz8_sb[:, gp * 256 : (gp + 1) * 256].rearrange(
                "p (k h) -> p k h", k=2
            )
        hlm_sb = lev.tile([128, 2, 4096], dt.float8e4, name="hlm", tag="hlm")
        wx_t = {}
        for l in (6, 5, 4):
            wx_t[l] = lev.tile(
                [128, 6, LVL_N[l]], dt.float8e4, name=f"wx{l}", tag=f"wx{l}"
            )
        for l in (3, 2, 1, 0):
            wx_t[l] = lev.tile(
                [128, 6, LVL_N[l]], dt.bfloat16, name=f"wx{l}", tag=f"wx{l}"
            )
        # h storage: a chunk's left-half h is written straight into the
        # parent level's wx tile block 5 (the pair-1 partner stream);
        # right-half h goes to hr_t[lvl].  c keeps per-level tiles.
        c_t = {}
        hr_t = {}
        for lvl in INT_LEVELS:
            n = LVL_N[lvl]
            hdt = dt.float32 if lvl == 0 else dt.bfloat16
            c_t[lvl] = lev.tile([H, n], hdt, name=f"c_l{lvl}", tag=f"c_l{lvl}")
            if lvl >= 1:
                hr_t[lvl] = lev.tile(
                    [H, n // 2], dt.bfloat16, name=f"hr{lvl}", tag=f"hr{lvl}"
                )
        h0_t = lev.tile([H, BL], dt.float32, name="h_l0", tag="h_l0")


        # ---------------- DMA schedule ----------------
        # sync HWDGE: mkb, weights, then hlm + wx7(u,i) in chunk-need order.
        # scalar HWDGE: wx7(o,hlu) pieces, then wx8 L6/L5/L4.
        # gpsimd SWDGE: only the small late tail wxd (descgen after the
        # broadcasts so the gpsimd engine is free when compute needs it).
        nc.sync.dma_start(out=ub8_sb, in_=ub8_d[:, :])
        nc.sync.dma_start(out=wm8_sb, in_=wm8_d[:, :])
        nc.scalar.dma_start(out=un8_sb, in_=un8_d[:, :])
        nc.scalar.dma_start(out=uz8_sb, in_=uz8_d[:, :])
        # mbx block: memset (kill NaN garbage: only rows 0-1 are real; the
        # paired stationary rows 2-127 are zero) then the (m7; ones) rows.
        # On GpSimd so the Vector warmup-feed memset is not delayed.
        nc.gpsimd.memset(l7s_sb[:, 1, :], 0.0)
        nc.sync.dma_start(out=l7s_sb[0:2, 1, :], in_=m1_d[:, :])

        # 1024-col pieces; group order matches SEQ: (7,0),(7,1) then (7,4),(7,5)
        # ... ; the 6 L7 streams are spread 2-per-queue across all 3 queues.
        for a in (0, 2048, 1024, 3072):
            b = a + 1024
            nc.sync.dma_start(out=hlm_sb[:, 0, a:b], in_=hlm_d[:, a:b])
            nc.sync.dma_start(out=hlm_sb[:, 1, a:b], in_=hlm_d[:, 4096 + a : 4096 + b])
            nc.scalar.dma_start(out=l7s_sb[:, 0, a:b], in_=l7s_d[:, a:b])
            nc.scalar.dma_start(
                out=l7s_sb[:, 2, a:b], in_=l7s_d[:, 4096 + a : 4096 + b]
            )
        # broadcast mask: not needed until L6 masks (~30us), after L7 pieces
        nc.sync.dma_start(out=mbc_sb[:, 0:2048], in_=mbc_d[:, 0:2048])
        nc.sync.dma_start(out=mbc_sb[:, 2048:MKCOLS], in_=mbc_d[:, 2048:MKCOLS])
        WX8_Q = {6: (0, 2), 5: (0, 1), 4: (0, 1, 2, 3, 4)}  # gates on sync
        for l in (6, 5, 4):
            n = LVL_N[l]
            base = WX8_OFF[l]
            for g in range(5):
                eng = nc.sync if g in WX8_Q[l] else nc.scalar
                eng.dma_start(
                    out=wx_t[l][:, g, :],
                    in_=wx8_d[:, base + g * n : base + (g + 1) * n],
                )

        # ---------------- PSUM + pools ----------------
        psum = ctx.enter_context(tc.tile_pool(name="psum", bufs=1, space="PSUM"))
        work = ctx.enter_context(tc.tile_pool(name="work", bufs=4))

        # PE warmup: fed by memset tile (no DMA dependency), ~8 cold matmuls
        # ~= 5us of activity so the HAM clock gate is open when work lands.
        warm_in = consts.tile([128, 512], dt.bfloat16)
        nc.vector.memset(warm_in, 0.0)
        warm = psum.tile([H, 512], dt.float32, tag="tU", name="warm")
        for _ in range(10):
            nc.tensor.matmul(warm, warm_in[:, 0:128], warm_in, start=True, stop=True)

        # tail wxd + bf16 weight pack DMAs (needed only from ~L3): their
        # SWDGE descgen runs on the gpsimd engine AFTER the broadcasts so
        # the masks are never blocked behind descriptor generation
        for l in (3, 2, 1, 0):
            n = LVL_N[l]
            base = WXI_OFF[l]
            for g in range(5):
                nc.gpsimd.dma_start(
                    out=wx_t[l][:, g, :],
                    in_=wxi_d[:, base + g * n : base + (g + 1) * n],
                )
        nc.gpsimd.dma_start(out=wts_sb, in_=wts_d[:, :])

        state = {}

        def phase_masks(lvl, j):
            if lvl == D - 1:
                return
            N = CW[lvl]
            c0 = j * N
            cch = c_t[lvl + 1]
            moff = MK_OFF[lvl] + c0
            mb = mbc_sb[:, moff : moff + N]
            hdt = dt.float8e4 if lvl in FP8_LEVELS else dt.bfloat16
            # gpsimd muls are slow (~1.1us); keep them only where the psum
            # runway covers them (big interleaved levels)
            eng2 = nc.gpsimd if lvl >= 5 else nc.vector

            h_e = wx_t[lvl][:, 5, c0 : c0 + N]       # child left-half h
            h_o = hr_t[lvl + 1][:, c0 : c0 + N]      # child right-half h
            hm = work.tile([128, 2, N], hdt, tag="hm", name="hm")
            nc.vector.tensor_mul(hm[:, 0, :], h_e, mb)
            eng2.tensor_mul(hm[:, 1, :], h_o, mb)
            cob = work.tile([128, N], dt.bfloat16, tag="cob", name="cob")
            eng2.tensor_mul(cob, cch[:, LVL_N[lvl] + c0 : LVL_N[lvl] + c0 + N], mb)
            state[(lvl, j)] = {
                "hm": hm,
                "cob": cob,
                "c_e": cch[:, c0 : c0 + N],
            }

        def phase_body(lvl, j):
            N = CW[lvl]
            c0 = j * N
            top = lvl == D - 1
            fp8 = lvl in FP8_LEVELS
            st = state.setdefault((lvl, j), {})
            odd = (lvl, j) in ODD_TAGS
            tagA = "tA" if odd else "bgA"
            tagB = "tB" if odd else "bgB"
            tagU = "tU" if odd else "bgu"
            if top and j % 2 == 1:
                # L7 uses only pA+pU; odd chunks ping-pong onto the banks
                # that pB/the tail set would otherwise leave idle here
                tagA, tagU = "bgB", "tU"

            gates = GATES_TOP if top else GATES_INT
            # pad psum tiles to full banks so odd/big sets never share a bank
            pA = psum.tile(
                [H, max(2 * N, 512)], dt.float32, tag=tagA, name=f"pA{lvl}_{j}"
            )[:, 0 : 2 * N]
            pB = (
                None
                if top
                else psum.tile(
                    [H, max(2 * N, 512)], dt.float32, tag=tagB, name=f"pB{lvl}_{j}"
                )[:, 0 : 2 * N]
            )
            pU = psum.tile([H, max(N, 512)], dt.float32, tag=tagU, name=f"pU{lvl}_{j}")[
                :, 0:N
            ]
            sl = {"u": pU}
            if top:
                sl["i"], sl["o"] = pA[:, 0:N], pA[:, N : 2 * N]
            else:
                sl["i"], sl["fl"] = pA[:, 0:N], pA[:, N : 2 * N]
                sl["fr"], sl["o"] = pB[:, 0:N], pB[:, N : 2 * N]

            def act(g):
                if g == "u":
                    gu = work.tile([128, N], dt.bfloat16, tag="gu", name="gu")
                    nc.scalar.activation(gu, pU, AF.Tanh)
                    st["gu"] = gu
                elif (top and g == "o") or (not top and g == "fl"):
                    gAB = work.tile([128, 2 * N], dt.bfloat16, tag="gAB", name="gAB")
                    nc.scalar.activation(gAB, pA, AF.Sigmoid)
                    if top:
                        st["gi"], st["go"] = gAB[:, 0:N], gAB[:, N : 2 * N]
                    else:
                        st["gi"], st["gfl"] = gAB[:, 0:N], gAB[:, N : 2 * N]
                elif not top and g == "o":
                    gFO = work.tile([128, 2 * N], dt.bfloat16, tag="gFO", name="gFO")
                    nc.scalar.activation(gFO, pB, AF.Sigmoid)
                    st["gfr"], st["go"] = gFO[:, 0:N], gFO[:, N : 2 * N]

            if fp8:
                # paired DoubleRow passes: (I;Uun_g)@(wxd_g;partner) then
                # (Ubt_g;Ubb_g)@(heb;hob)
                if top:
                    hm = hlm_sb[:, :, c0 : c0 + N]
                    gpos = {"u": 0, "i": 1, "o": 2}
                else:
                    T = 6
                    wxt = wx_t[lvl]
                    hm = st["hm"]
                    gpos = G_POS
                def pair1(g):
                    if top:
                        nc.tensor.matmul(
                            sl[g], wm8g(gpos[g]), l7s_sb[:, 0:2, c0 : c0 + N],
                            start=True, stop=False, perf_mode=DR,
                        )
                        nc.tensor.matmul(
                            sl[g], uz8g(gpos[g]), l7s_sb[:, 1:3, c0 : c0 + N],
                            start=False, stop=False, perf_mode=DR,
                        )
                        return
                    gp = gpos[g]
                    pair = wxt[:, gp : T : T - 1 - gp, c0 : c0 + N]
                    nc.tensor.matmul(
                        sl[g], un8g(g), pair, start=True, stop=False,
                        perf_mode=DR,
                    )

                def pair2(g):
                    nc.tensor.matmul(
                        sl[g], ub8g(g), hm, start=False, stop=True,
                        perf_mode=DR,
                    )
                    act(g)

                if N == 512:
                    # gate slices are bank-aligned: full runway legal
                    for g in gates:
                        pair1(g)
                    for g in gates:
                        pair2(g)
                else:
                    # pA/pB slices share a bank: emit bank-disjoint waves
                    for grp in (("u", "i"), ("fl", "fr"), ("o",)):
                        for g in grp:
                            pair1(g)
                        for g in grp:
                            pair2(g)
            else:
                hm = st["hm"]
                heb, hob = hm[:, 0, :], hm[:, 1, :]
                h_e = wx_t[lvl][:, 5, c0 : c0 + N]
                wxs = {g: wx_t[lvl][:, G_POS[g], c0 : c0 + N] for g in gates}
                # tail chunks share PSUM banks between gate slices -> strictly
                # gate-major (one open accumulation group per bank at a time)
                for g in gates:
                    ps = sl[g]
                    nc.tensor.matmul(ps, eye, wxs[g], start=True, stop=False)
                    nc.tensor.matmul(ps, ubt(g), heb, start=False, stop=False)
                    last = G_UUN[g] is None
                    nc.tensor.matmul(ps, ubb(g), hob, start=False, stop=last)
                    if not last:
                        nc.tensor.matmul(ps, uun(g), h_e, start=False, stop=True)
                    act(g)

        def phase_chain(lvl, j):
            N = CW[lvl]
            c0 = j * N
            top = lvl == D - 1
            st = state.pop((lvl, j))
            cs = c_t[lvl][:, c0 : c0 + N]
            wdt = dt.float32 if lvl == 0 else dt.bfloat16
            if top:
                nc.vector.tensor_mul(cs, st["gi"], st["gu"])
            else:
                t1 = work.tile([128, N], wdt, tag="t1", name="t1")
                nc.vector.tensor_mul(t1, st["gi"], st["gu"])
                t2 = work.tile([128, N], wdt, tag="t2", name="t2")
                nc.vector.tensor_mul(t2, st["gfl"], st["c_e"])
                t3 = work.tile([128, N], wdt, tag="t3", name="t3")
                nc.vector.tensor_mul(t3, st["gfr"], st["cob"])
                nc.vector.tensor_add(cs, t1, t2)
                nc.vector.tensor_add(cs, cs, t3)
            tch = work.tile([128, N], wdt, tag="tch", name="tch")
            nc.scalar.activation(tch, cs, AF.Tanh)
            # h destinations: left-half columns -> parent wx block 5
            # (pair-1 partner); right-half -> hr_t[lvl]; root -> h0
            if lvl == 0:
                nc.vector.tensor_mul(h0_t[:, c0 : c0 + N], st["go"], tch)
            else:
                half = LVL_N[lvl] // 2
                if c0 + N <= half:
                    nc.vector.tensor_mul(
                        wx_t[lvl - 1][:, 5, c0 : c0 + N], st["go"], tch
                    )
                elif c0 >= half:
                    nc.vector.tensor_mul(
                        hr_t[lvl][:, c0 - half : c0 - half + N], st["go"], tch
                    )
                else:
                    nl = half - c0
                    nc.vector.tensor_mul(
                        wx_t[lvl - 1][:, 5, c0 : c0 + nl],
                        st["go"][:, 0:nl], tch[:, 0:nl],
                    )
                    nc.vector.tensor_mul(
                        hr_t[lvl][:, 0 : N - nl],
                        st["go"][:, nl:N], tch[:, nl:N],
                    )

        pending = []
        for seq_i, (lvl, j) in enumerate(SEQ):
            if seq_i == 2:
                # keep-warm burst: the first L7 chunks finish ~3-7us before
                # the next pieces land; without activity the HAM clock gate
                # drops the PE back to 1.2GHz mid-level
                warm2 = psum.tile([H, 512], dt.float32, tag="tU", name="warm2")
                for _ in range(6):
                    nc.tensor.matmul(
                        warm2, warm_in[:, 0:128], warm_in, start=True, stop=True
                    )
            for ch in _children(lvl, j):
                if ch in pending:
                    phase_chain(*ch)
                    pending.remove(ch)
            phase_masks(lvl, j)
            phase_body(lvl, j)
            pending.append((lvl, j))
            while len(pending) > 1:
                phase_chain(*pending.pop(0))
        for ch in pending:
            phase_chain(*ch)

        nc.sync.dma_start(out=h_out_d[:, :], in_=h0_t[:, :BL])
        nc.scalar.dma_start(out=c_out_d[:, :], in_=c_t[0][:, :BL])

    nc.finalize()
    _CACHE["nc"] = nc
    return nc


def prep_shared_inputs(emb, W, bW, Ubin, bUbin, Uun, bUun):
    import concourse.mybir as mybir

    F8 = np.dtype(mybir.dt.np(mybir.dt.float8e4))

    emb = np.asarray(emb, np.float32)
    W = np.asarray(W, np.float32)
    bW = np.asarray(bW, np.float32)
    Ubin = np.asarray(Ubin, np.float32)
    bUbin = np.asarray(bUbin, np.float32)
    Uun = np.asarray(Uun, np.float32)
    bUun = np.asarray(bUun, np.float32)

    # gate order u, i, fl, fr, o ; bias b (unary) and b+d (binary)
    b_rows = np.stack(
        [
            bW[3] + bUun[3],      # u
            bW[0] + bUun[0],      # i
            bW[1] + bUun[1],      # fl
            bW[1] + bUbin[2],     # fr (same either arity; unary killed via cob)
            bW[2] + bUun[2],      # o
        ]
    )
    bd_rows = np.stack(
        [
            bW[3] + bUbin[4],
            bW[0] + bUbin[0],
            bW[1] + bUbin[1],
            bW[1] + bUbin[2],
            bW[2] + bUbin[3],
        ]
    )
    Wg = np.stack([W[3], W[0], W[1], W[1], W[2]])  # u,i,fl,fr,o

    # [2V, 5, H] combined tables indexed by tok + m*V (m=1 -> binary biases)
    wx = np.einsum("ve,geh->vgh", emb, Wg, optimize=True)
    tab_bf = np.empty((2 * V, 5, H), dtype=BF16)
    tab_bf[:V] = (wx + b_rows[None, :, :]).astype(BF16)
    tab_bf[V:] = (wx + bd_rows[None, :, :]).astype(BF16)
    tab_f8 = tab_bf.astype(np.float32).astype(F8)

    hleaf_tab = np.tanh(emb @ W[3] + bW[3]).astype(F8)
    emb_f8 = emb.astype(F8)

    # Ubt' = Ubt - Uun so the unary term can use UNMASKED child h:
    # binary cols:  Ubt'@h + Uun@h = Ubt@h ;  unary cols: Ubt'@0 + Uun@h
    ub_order = [4, 0, 1, 2, 3]  # Ubin gate index for u,i,fl,fr,o
    uun_of = [Uun[3], Uun[0], Uun[1], None, Uun[2]]
    ubt_p = np.concatenate(
        [
            Ubin[g][:128] - (u if u is not None else 0)
            for g, u in zip(ub_order, uun_of)
        ],
        axis=1,
    )
    ubb_p = np.concatenate([Ubin[g][128:] for g in ub_order], axis=1)
    uun_p = np.concatenate([Uun[g] for g in range(4)], axis=1)
    eye = np.eye(128, dtype=np.float32)
    wts = np.concatenate([ubt_p, ubb_p, uun_p, eye], axis=1).astype(BF16)

    # fp8 pair packs: per gate (I | Uun_g) and (Ubt_g | Ubb_g), [128, 5*2*128]
    uun_g = {"u": Uun[3], "i": Uun[0], "fl": Uun[1], "fr": np.zeros_like(eye),
             "o": Uun[2]}
    un8 = np.concatenate(
        [np.concatenate([eye, uun_g[g]], axis=1) for g in GATES_INT], axis=1
    ).astype(F8)
    ub8 = np.concatenate(
        [
            np.concatenate(
                [Ubin[gi][:128] - (u if u is not None else 0), Ubin[gi][128:]],
                axis=1,
            )
            for gi, u in zip(ub_order, uun_of)
        ],
        axis=1,
    ).astype(F8)

    # L7 pair stationaries for gates u,i,o: (W_g | M_g) and (0 | Uun_g),
    # where M_g rows 0/1 are the arity delta d_g and unary bias b_g
    wm_parts = []
    uz_parts = []
    for gi, wg, ug in ((0, W[3], Uun[3]), (1, W[0], Uun[0]), (4, W[2], Uun[2])):
        M = np.zeros((128, 128), np.float32)
        M[0] = bd_rows[gi] - b_rows[gi]
        M[1] = b_rows[gi]
        wm_parts.append(np.concatenate([Wg[gi], M], axis=1))
        uz_parts.append(
            np.concatenate([np.zeros((128, 128), np.float32), ug], axis=1)
        )
    wm8 = np.concatenate(wm_parts, axis=1).astype(F8)
    uz8 = np.concatenate(uz_parts, axis=1).astype(F8)

    return dict(
        _tab_bf=tab_bf, _tab_f8=tab_f8, _hleaf=hleaf_tab, _f8=F8, _emb8=emb_f8,
        wts=np.ascontiguousarray(wts), un8=np.ascontiguousarray(un8),
        ub8=np.ascontiguousarray(ub8), wm8=np.ascontiguousarray(wm8),
        uz8=np.ascontiguousarray(uz8),
    )


def prep_core_inputs(tokens_c, arity_c, shared):
    """Per-core inputs: gather vocab tables into feature-major streams.

    Each level is packed in bit-reversed node order, trees fastest
    (col = position * BL + tree).
    """
    tokens_c = np.asarray(tokens_c)
    arity_c = np.asarray(arity_c, np.int64)
    tab_bf = shared["_tab_bf"]
    tab_f8 = shared["_tab_f8"]
    hleaf_tab = shared["_hleaf"]
    F8 = shared["_f8"]

    def lvl_toks(l):
        off = 2**l - 1
        return tokens_c[:, off + SIG[l]].T.reshape(-1)

    def lvl_mask(l):
        off = 2**l - 1
        return (arity_c[:, off + SIG[l]].T.reshape(-1) == 1)

    # L7 streams: raw emb x7 | raw hleaf_l ; masked hlm_l/hlm_r; m1 rows
    t7 = lvl_toks(7)
    m7 = lvl_mask(7)
    leaf_toks = lvl_toks(8)
    hl = hleaf_tab[leaf_toks]  # [8192, H] fp8
    hl_l, hl_r = hl[:4096], hl[4096:]
    m7f = m7[:, None]
    z8 = np.zeros((1, 1), dtype=F8)
    l7s = np.concatenate([shared["_emb8"][t7].T, hl_l.T], axis=1)
    m1 = np.stack([m7.astype(np.float32), np.ones(4096, np.float32)]).astype(F8)
    hlm = np.concatenate(
        [np.where(m7f, hl_l, z8).T, np.where(m7f, hl_r, z8).T], axis=1
    )

    # internal levels: fp8 for 6..4, bf16 for 3..0
    wx8_cols = []
    wxi_cols = []
    mrows = []
    for l in range(6, -1, -1):
        tl = lvl_toks(l)
        ml = lvl_mask(l)
        if l >= 4:
            blk = tab_f8[tl + ml * V].transpose(1, 2, 0)  # [5, H, N_l]
            wx8_cols.append(np.concatenate(list(blk), axis=1))
        else:
            blk = tab_bf[tl + ml * V].transpose(1, 2, 0)
            wxi_cols.append(np.concatenate(list(blk), axis=1))
        mrows.append(ml.astype(BF16))
    wx8 = np.ascontiguousarray(np.concatenate(wx8_cols, axis=1))
    wxi = np.ascontiguousarray(np.concatenate(wxi_cols, axis=1))
    mrow = np.concatenate(mrows).astype(np.float32)

    out = {k: v for k, v in shared.items() if not k.startswith("_")}
    out.update(
        l7s=np.ascontiguousarray(l7s),
        m1=np.ascontiguousarray(m1),
        hlm=np.ascontiguousarray(hlm),
        wx8=wx8,
        wxi=wxi,
        mbc=np.ascontiguousarray(np.broadcast_to(mrow, (128, MKCOLS)).astype(F8)),
    )
    return out


def kernel(tokens, arity, emb, W, bW, Ubin, bUbin, Uun, bUun):
    from concourse.bass_utils import run_bass_kernel_spmd

    tokens = np.asarray(tokens)
    arity = np.asarray(arity)

    shared = prep_shared_inputs(emb, W, bW, Ubin, bUbin, Uun, bUun)
    in_maps = [
        prep_core_inputs(
            tokens[k * BL : (k + 1) * BL], arity[k * BL : (k + 1) * BL], shared
        )
        for k in range(NCORES)
    ]

    nc = _build_nc()
    res = run_bass_kernel_spmd(nc, in_maps, core_ids=list(range(NCORES)))
    results = res.results

    h = np.concatenate([r["h_out"].T for r in results], axis=0)
    c = np.concatenate([r["c_out"].T for r in results], axis=0)
    return h.astype(np.float32), c.astype(np.float32)
